# revision 44
# baseline (speedup 1.0000x reference)
"""Trainium2 Bass kernel for the AttentionHook module.

Math (per batch b, N = H*W = 4096):
    f = wq @ x   [N];   g = wk @ x   [N];   h = wv @ x   [C, N]
    scores[i, j] = f[i] * g[j]      (rank-1 outer product!)
    beta = softmax(scores, axis=0)  (normalize over i, per column j)
    o = (1-gamma) * h @ beta + gamma * x
Because scores are rank-1, quantize f onto a uniform grid of L=96 levels
(f = fhat + eps) and bucket by level; first-order eps correction:
    sum_n h[c,n] e^{f_n g_m} ~= sum_lev e^{fhat_lev g_m} (H0 + g_m H1)[c,lev]

Key structural choices vs the v0 kernel:
  * bucket RAW x (shipped n-major as x^T chunks straight from DMA) and
    apply wv AFTER bucketing: H0 = wv @ X0 — turns the [C, N] h-compute
    plus its PSUM->SBUF staging into a tiny [C, L] transform.
  * f, g, idx, eps, cnt, Seps are exact fp32 on the host (2 matvecs +
    bincounts, the same scale of host work as the final divide). g is
    re-broadcast on-device across the 96 level partitions with
    16-partition selector matmuls against a bf16 [gh; gl] stack so the
    hi+lo split of g sums exactly in PSUM.
  * PE p-state is kept hot with dependency-free warm matmuls during the
    input-DMA wait and the bucket->main transform bubble.

Per core (one batch per core, 8 cores):
  bcast: pg[lev, m] = g_m (f32-accurate) via sel^T @ [gh; gl] matmuls.
  exp:   E = exp(pg * fgrid) on ScalarE; Eg = E * pg on DVE.
  masks: one-hot (iota == idx) on GpSimd; meb = mask*eps on DVE.
  bucket: X0/X1 += mask^T @ x^T per 128-chunk (TensorE).
  transform: Xk -> SBUF bf16, 2 TensorE transposes each, Hk = Xk^T @ wv^T.
  main:  po[m, 0:257] = E^T @ [H0|cnt] + Eg^T @ [H1|Seps] (PSUM accum,
         hb1 term lagged so the first matmuls only wait on H0).
  out:   ship [num | Z] bf16 chunk-major; host divides + transposes.
"""

import numpy as np
from contextlib import ExitStack

B, C, HH, WW = 8, 256, 64, 64
N = HH * WW            # 4096
P = 128
NCH = N // P           # 32 n-chunks (also m-chunks)
L = 96                 # f-quantization levels
FRNG = 6.0             # f grid covers [-6, 6)
DELTA = 2 * FRNG / L   # 0.125
OW = C + 1             # output row width: [num(256) | Z]
XTW = NCH * C          # 8192 bf16 cols in the x^T blob

# aux blob (bf16 cols; f32 regions live in the first 512 bf16 cols)
#   f32 view cols: iota 0:96 | idx 96:128 | eps 128:160 | fgrid 160 |
#                  cnt 161 | Seps 162
A_ONES = 512           # all-ones [2, 96] for the g-broadcast matmuls
A_WVT = 608            # wv^T c-chunks [128, 2*256]
A_IDEN = 1120          # identity [128, 128]
AUXW = 1248

_CACHE = {}


def _build():
    import concourse.tile as tile
    from concourse import bacc, mybir

    f32 = mybir.dt.float32
    bf16 = mybir.dt.bfloat16
    Exp = mybir.ActivationFunctionType.Exp
    Alu = mybir.AluOpType

    nc = bacc.Bacc("TRN2", target_bir_lowering=False, debug=False)
    xt_d = nc.dram_tensor("xt", [P, XTW], bf16, kind="ExternalInput").ap()
    aux_d = nc.dram_tensor("aux", [P, AUXW], bf16, kind="ExternalInput").ap()
    gstk_d = nc.dram_tensor("gstk", [2, N], bf16, kind="ExternalInput").ap()
    o_d = nc.dram_tensor("o", [P, NCH * OW], bf16, kind="ExternalOutput").ap()

    with tile.TileContext(nc) as tc, ExitStack() as ctx:
        cpool = ctx.enter_context(tc.tile_pool(name="cpool", bufs=1))
        xt_sb = cpool.tile([P, XTW], bf16, tag="xt", name="xt_sb")
        aux_sb = cpool.tile([P, AUXW], bf16, tag="aux", name="aux_sb")
        auxf = aux_sb[:, 0:512].bitcast(f32)      # [128, 256] f32 view
        iota_sb = auxf[:, 0:L]
        idx_sb = auxf[:, L:L + NCH]
        eps_sb = auxf[:, L + NCH:L + 2 * NCH]
        fgrid_sb = auxf[:, 160:161]
        cnt_sb = auxf[:, 161:162]
        seps_sb = auxf[:, 162:163]
        ones_sb = aux_sb[:, A_ONES:A_ONES + L]    # rows 0:2 used
        wvt_sb = aux_sb[:, A_WVT:A_WVT + 512]     # [128, 2*256]
        iden_sb = aux_sb[:, A_IDEN:A_IDEN + P]
        gstk_sb = cpool.tile([2, N], bf16, tag="gstk", name="gstk_sb")

        e_sb = cpool.tile([L, N], bf16, tag="e", name="e_sb")
        eg_sb = cpool.tile([L, N], bf16, tag="eg", name="eg_sb")
        xb0_sb = cpool.tile([L, C], bf16, tag="xb0", name="xb0_sb")
        xb1_sb = cpool.tile([L, C], bf16, tag="xb1", name="xb1_sb")
        x0t_sb = cpool.tile([P, 2, L], bf16, tag="x0t", name="x0t_sb")
        x1t_sb = cpool.tile([P, 2, L], bf16, tag="x1t", name="x1t_sb")
        hb0_sb = cpool.tile([L, OW], bf16, tag="hb0", name="hb0_sb")
        hb1_sb = cpool.tile([L, OW], bf16, tag="hb1", name="hb1_sb")

        # ---- input DMA. Transfers sharing a queue round-robin across its
        # DMA engines and complete together, so aux (which gates phase 1 +
        # masks) rides ALONE on sync, the earliest-starting queue. A tiny
        # ordering DMA then holds cut0's descriptors back until aux is
        # done, so cut0 still gets the sync queue to itself afterwards.
        # Cut sizes are matched to the measured queue start times
        # (sync ~8.7us, scalar ~10.2us, gpsimd ~12.4us).
        dummy_sb = cpool.tile([1, 2], bf16, tag="dummy", name="dummy_sb")
        nc.sync.dma_start(aux_sb[:], aux_d[:, :])
        nc.sync.dma_start(dummy_sb[0:1, :], aux_sb[0:1, 0:2])
        nc.sync.dma_start(xt_sb[:, 0:8 * C], xt_d[:, 0:8 * C])
        nc.scalar.dma_start(xt_sb[:, 8 * C:20 * C], xt_d[:, 8 * C:20 * C])
        nc.scalar.dma_start(gstk_sb[:], gstk_d[:, :])
        nc.gpsimd.dma_start(xt_sb[:, 20 * C:], xt_d[:, 20 * C:])

        # warm tile: no input deps -> PE ramps p-state during input DMA
        warm_sb = cpool.tile([P, 512], bf16, tag="warm", name="warm_sb")
        nc.gpsimd.memset(warm_sb[:], 0.0)

        bctx = ExitStack()
        pgp = bctx.enter_context(tc.tile_pool(name="pgp", bufs=3, space="PSUM"))
        psbp = bctx.enter_context(tc.tile_pool(name="psbp", bufs=1, space="PSUM"))
        mkp = bctx.enter_context(tc.tile_pool(name="mkp", bufs=2))
        psb0 = psbp.tile([L, C], f32, tag="psb0", name="psb0")
        psb1 = psbp.tile([L, C], f32, tag="psb1", name="psb1")

        def warm(i, k):
            # shares the pg pool/tag so warm-up costs no extra PSUM bank
            pw = pgp.tile([L, 512], f32, tag="pg", name=f"pw{i}")
            for j in range(k):
                nc.tensor.matmul(pw[:], warm_sb[:, 0:L], warm_sb[:],
                                 start=(j == 0), stop=(j == k - 1))

        warm(0, 3)

        def xtc(n):
            return xt_sb[:, n * C:(n + 1) * C]

        pg_t = {}

        def bcast_exp(q):
            # pg[lev, 512q:512q+512] = g (exact: gh+gl sum in PSUM)
            lo = q * 512
            pg = pgp.tile([L, 512], f32, tag="pg", name=f"pg{q}")
            pg_t[q] = pg
            nc.tensor.matmul(pg[:], ones_sb[0:2, :],
                             gstk_sb[0:2, lo:lo + 512], start=True, stop=True)
            nc.scalar.activation(e_sb[:, lo:lo + 512], pg[:], Exp,
                                 scale=fgrid_sb[0:L, :])

        def eg4(q):
            # Eg = E * g; emitted after every is_eq so the in-order DVE
            # queue never stalls a mask the buckets are waiting on
            lo = q * 512
            pg = pg_t.pop(q)
            nc.vector.tensor_tensor(eg_sb[:, lo:lo + 512], e_sb[:, lo:lo + 512],
                                    pg[:], Alu.mult)

        def masks4(q):
            # one-hot masks for 4 chunks: mkb[p, j, lev] = (iota == idx)
            mkb = mkp.tile([P, 4, L], bf16, tag="mkb", name=f"mkb{q}")
            meb = mkp.tile([P, 4, L], bf16, tag="meb", name=f"meb{q}")
            iota3 = iota_sb.unsqueeze(1).broadcast_to([P, 4, L])
            idx3 = idx_sb[:, 4 * q:4 * q + 4].unsqueeze(2).broadcast_to(
                [P, 4, L])
            eps3 = eps_sb[:, 4 * q:4 * q + 4].unsqueeze(2).broadcast_to(
                [P, 4, L])
            nc.vector.tensor_tensor(mkb[:, :, :], iota3, idx3, Alu.is_equal)
            nc.gpsimd.tensor_mul(meb[:, :, :], mkb[:, :, :], eps3)
            return mkb, meb

        def buckets4(q, mkb, meb):
            for n in range(4 * q, 4 * q + 4):
                j = n % 4
                nc.tensor.matmul(psb0[:], mkb[:, j, :], xtc(n),
                                 start=(n == 0), stop=(n == NCH - 1))
                nc.tensor.matmul(psb1[:], meb[:, j, :], xtc(n),
                                 start=(n == 0), stop=(n == NCH - 1))

        # cnt/Seps columns of the H blocks come straight from the host
        nc.vector.tensor_copy(hb0_sb[:, C:C + 1], cnt_sb[0:L, :])
        nc.vector.tensor_copy(hb1_sb[:, C:C + 1], seps_sb[0:L, :])

        # All is_eq masks queue on DVE before any eg. Buckets are the
        # early phase (gated only by aux + xt cuts); bcast/exp/eg are
        # needed only by the main phase and slot between bucket groups
        # (pgp bufs=3 -> bcast q+3 needs eg(q), placed just ahead of it).
        # Warm matmuls fill the DMA-wait stalls so the PE p-state stays
        # ramped through the bucket phase.
        for q in range(8):
            mkb, meb = masks4(q)   # is_eq first: masks gate the buckets,
            bcast_exp(q)           # eg is only needed by the main phase
            eg4(q)
            buckets4(q, mkb, meb)
            if q == 6:
                # fill the tail DMA-wait so the PE p-state is ramped for
                # the last bucket group + transform + main phases
                warm(2, 6)

        # ---- transform: H = wv @ X (via TensorE transposes of X)
        ptp = bctx.enter_context(tc.tile_pool(name="ptp", bufs=1, space="PSUM"))
        phbp = bctx.enter_context(tc.tile_pool(name="phbp", bufs=1,
                                               space="PSUM"))
        warm(1, 5)  # keep clocks hot through the copy-bound bubble

        # both transform chains interleaved so scalar/vector/PE pipeline
        nc.scalar.copy(xb0_sb[:, 0:P], psb0[:, 0:P])
        nc.vector.tensor_copy(xb0_sb[:, P:C], psb0[:, P:C])
        nc.scalar.copy(xb1_sb[:, 0:P], psb1[:, 0:P])
        nc.vector.tensor_copy(xb1_sb[:, P:C], psb1[:, P:C])
        for cc in range(2):
            for (i, (xb, xtt)) in enumerate(((xb0_sb, x0t_sb),
                                             (xb1_sb, x1t_sb))):
                pt = ptp.tile([P, L], bf16, tag="pt", name=f"pt{i}{cc}")
                nc.tensor.transpose(pt[:], xb[0:L, cc * P:(cc + 1) * P],
                                    iden_sb[0:L, 0:L])
                if cc == 0:
                    nc.scalar.copy(xtt[:, cc, :], pt[:])
                else:
                    nc.vector.tensor_copy(xtt[:, cc, :], pt[:])
        phb0 = phbp.tile([L, C], f32, tag="phb0", name="phb0")
        phb1 = phbp.tile([L, C], f32, tag="phb1", name="phb1")
        for cc in range(2):
            nc.tensor.matmul(phb0[:], x0t_sb[:, cc, :],
                             wvt_sb[:, cc * C:(cc + 1) * C],
                             start=(cc == 0), stop=(cc == 1))
        for cc in range(2):
            nc.tensor.matmul(phb1[:], x1t_sb[:, cc, :],
                             wvt_sb[:, cc * C:(cc + 1) * C],
                             start=(cc == 0), stop=(cc == 1))
        nc.scalar.copy(hb0_sb[:, 0:P], phb0[:, 0:P])
        nc.vector.tensor_copy(hb0_sb[:, P:C], phb0[:, P:C])
        nc.scalar.copy(hb1_sb[:, 0:P], phb1[:, 0:P])
        nc.vector.tensor_copy(hb1_sb[:, P:C], phb1[:, P:C])
        bctx.close()

        # ---- main: po = E^T @ [H0|cnt] + Eg^T @ [H1|Seps]; the hb1 term
        # lags so the first matmuls start as soon as hb0 lands.
        OBAT = 4
        LAG = 5
        with tc.tile_pool(name="pop", bufs=8, space="PSUM") as pop, \
             tc.tile_pool(name="otp", bufs=4) as otp:
            pos = {}
            ot = None
            for t in range(NCH + LAG):
                if t < NCH:
                    po = pop.tile([P, OW], f32, tag="po", name=f"po{t}")
                    pos[t] = po
                    nc.tensor.matmul(po[:], e_sb[:, t * P:(t + 1) * P],
                                     hb0_sb[:], start=True, stop=False)
                if t >= LAG:
                    mc = t - LAG
                    po2 = pos.pop(mc)
                    nc.tensor.matmul(po2[:], eg_sb[:, mc * P:(mc + 1) * P],
                                     hb1_sb[:], start=False, stop=True)
                    k = mc % OBAT
                    if k == 0:
                        ot = otp.tile([P, OBAT * OW], bf16, tag="ot",
                                      name=f"ot{mc // OBAT}")
                    dst = ot[:, k * OW:(k + 1) * OW]
                    if mc % 2 == 0:
                        nc.scalar.copy(dst, po2[:])
                    else:
                        nc.vector.tensor_copy(dst, po2[:])
                    if k == OBAT - 1:
                        ob = mc // OBAT
                        c0 = ob * OBAT * OW
                        oq = nc.sync if ob % 2 == 0 else nc.gpsimd
                        oq.dma_start(o_d[:, c0:c0 + OBAT * OW], ot[:])

    nc.compile()
    return nc


def _get_nc():
    if "nc" not in _CACHE:
        _CACHE["nc"] = _build()
    return _CACHE["nc"]


def make_in_maps(x, wq, wk, wv):
    import ml_dtypes
    bf = ml_dtypes.bfloat16
    xf = np.ascontiguousarray(x, dtype=np.float32).reshape(B, C, N)
    wq = np.asarray(wq, dtype=np.float32).reshape(C)
    wk = np.asarray(wk, dtype=np.float32).reshape(C)
    wv = np.asarray(wv, dtype=np.float32)

    fgrid = np.arange(L, dtype=np.float32) * DELTA - FRNG

    in_maps = []
    for b in range(B):
        xb = xf[b]                                   # [C, N]
        f = wq @ xb                                  # [N] exact fp32
        g = wk @ xb
        idx = np.clip(np.round((f + FRNG) / DELTA), 0, L - 1).astype(
            np.int64)
        eps = f - fgrid[idx]
        cnt = np.bincount(idx, minlength=L).astype(np.float32)
        seps = np.bincount(idx, weights=eps.astype(np.float64),
                           minlength=L).astype(np.float32)

        xt = np.ascontiguousarray(
            xb.T.reshape(NCH, P, C).transpose(1, 0, 2).reshape(P, XTW)
        ).astype(bf)

        gh = g.astype(bf)
        gl = (g - gh.astype(np.float32)).astype(bf)
        gstk = np.stack([gh, gl])                     # [2, N]

        aux = np.zeros((P, AUXW), dtype=bf)
        auxf = aux[:, 0:512].view(np.float32)
        auxf[:, 0:L] = np.arange(L, dtype=np.float32)[None, :]
        auxf[:, L:L + NCH] = idx.astype(np.float32).reshape(NCH, P).T
        auxf[:, L + NCH:L + 2 * NCH] = eps.reshape(NCH, P).T
        auxf[0:L, 160] = fgrid
        auxf[0:L, 161] = cnt
        auxf[0:L, 162] = seps
        aux[0:2, A_ONES:A_ONES + L] = bf(1.0)
        aux[:, A_WVT:A_WVT + C] = wv[:, 0:P].T.astype(bf)
        aux[:, A_WVT + C:A_WVT + 2 * C] = wv[:, P:2 * P].T.astype(bf)
        aux[:, A_IDEN:A_IDEN + P] = np.eye(P, dtype=np.float32).astype(bf)

        in_maps.append({"xt": xt, "aux": aux, "gstk": gstk})
    return in_maps, xf


def kernel(x, wq, wk, wv, gamma):
    from concourse.bass_utils import run_bass_kernel_spmd

    in_maps, xf = make_in_maps(x, wq, wk, wv)
    nc = _get_nc()
    res = run_bass_kernel_spmd(nc, in_maps, core_ids=list(range(B)))

    g0 = float(np.asarray(gamma, dtype=np.float32).reshape(-1)[0])
    out = np.empty((B, C, HH, WW), dtype=np.float32)
    for b in range(B):
        onz = res.results[b]["o"].astype(np.float32)  # [P, NCH*257]
        onz = onz.reshape(P, NCH, OW).transpose(1, 0, 2).reshape(N, OW)
        o = (onz[:, 0:C] / onz[:, C:C + 1]).T         # [C, N]
        if g0 != 0.0:
            o = (1.0 - g0) * o + g0 * xf[b]
        out[b] = o.reshape(C, HH, WW)
    return out


# revision 46
# speedup vs baseline: 1.0267x; 1.0267x over previous
"""Trainium2 Bass kernel for the AttentionHook module.

Math (per batch b, N = H*W = 4096):
    f = wq @ x   [N];   g = wk @ x   [N];   h = wv @ x   [C, N]
    scores[i, j] = f[i] * g[j]      (rank-1 outer product!)
    beta = softmax(scores, axis=0)  (normalize over i, per column j)
    o = (1-gamma) * h @ beta + gamma * x
Because scores are rank-1, quantize f onto a uniform grid of L=96 levels
(f = fhat + eps) and bucket by level; first-order eps correction:
    sum_n h[c,n] e^{f_n g_m} ~= sum_lev e^{fhat_lev g_m} (H0 + g_m H1)[c,lev]

Key structural choices vs the v0 kernel:
  * bucket RAW x (shipped n-major as x^T chunks straight from DMA) and
    apply wv AFTER bucketing: H0 = wv @ X0 — turns the [C, N] h-compute
    plus its PSUM->SBUF staging into a tiny [C, L] transform.
  * f, g, idx, eps, cnt, Seps are exact fp32 on the host (2 matvecs +
    bincounts, the same scale of host work as the final divide). g is
    re-broadcast on-device across the 96 level partitions with
    16-partition selector matmuls against a bf16 [gh; gl] stack so the
    hi+lo split of g sums exactly in PSUM.
  * PE p-state is kept hot with dependency-free warm matmuls during the
    input-DMA wait and the bucket->main transform bubble.

Per core (one batch per core, 8 cores):
  bcast: pg[lev, m] = g_m (f32-accurate) via sel^T @ [gh; gl] matmuls.
  exp:   E = exp(pg * fgrid) on ScalarE; Eg = E * pg on DVE.
  masks: one-hot (iota == idx) on GpSimd; meb = mask*eps on DVE.
  bucket: X0/X1 += mask^T @ x^T per 128-chunk (TensorE).
  transform: Xk -> SBUF bf16, 2 TensorE transposes each, Hk = Xk^T @ wv^T.
  main:  po[m, 0:257] = E^T @ [H0|cnt] + Eg^T @ [H1|Seps] (PSUM accum,
         hb1 term lagged so the first matmuls only wait on H0).
  out:   ship [num | Z] bf16 chunk-major; host divides + transposes.
"""

import numpy as np
from contextlib import ExitStack

B, C, HH, WW = 8, 256, 64, 64
N = HH * WW            # 4096
P = 128
NCH = N // P           # 32 n-chunks (also m-chunks)
L = 96                 # f-quantization levels
FRNG = 6.0             # f grid covers [-6, 6)
DELTA = 2 * FRNG / L   # 0.125
OW = C + 1             # output row width: [num(256) | Z]
XTW = NCH * C          # 8192 bf16 cols in the x^T blob

# aux blob (bf16 cols; f32 regions live in the first 512 bf16 cols)
#   f32 view cols: iota 0:96 | idx 96:128 | eps 128:160 | fgrid 160 |
#                  cnt 161 | Seps 162
A_ONES = 512           # all-ones [2, 96] for the g-broadcast matmuls
A_WVT = 608            # wv^T c-chunks [128, 2*256]
A_IDEN = 1120          # identity [128, 128]
AUXW = 1248

_CACHE = {}


def _build():
    import concourse.tile as tile
    from concourse import bacc, mybir

    f32 = mybir.dt.float32
    bf16 = mybir.dt.bfloat16
    Exp = mybir.ActivationFunctionType.Exp
    Alu = mybir.AluOpType

    nc = bacc.Bacc("TRN2", target_bir_lowering=False, debug=False)
    xt_d = nc.dram_tensor("xt", [P, XTW], bf16, kind="ExternalInput").ap()
    aux_d = nc.dram_tensor("aux", [P, AUXW], bf16, kind="ExternalInput").ap()
    gstk_d = nc.dram_tensor("gstk", [2, N], bf16, kind="ExternalInput").ap()
    o_d = nc.dram_tensor("o", [P, NCH * OW], bf16, kind="ExternalOutput").ap()

    with tile.TileContext(nc) as tc, ExitStack() as ctx:
        cpool = ctx.enter_context(tc.tile_pool(name="cpool", bufs=1))
        xt_sb = cpool.tile([P, XTW], bf16, tag="xt", name="xt_sb")
        aux_sb = cpool.tile([P, AUXW], bf16, tag="aux", name="aux_sb")
        auxf = aux_sb[:, 0:512].bitcast(f32)      # [128, 256] f32 view
        iota_sb = auxf[:, 0:L]
        idx_sb = auxf[:, L:L + NCH]
        eps_sb = auxf[:, L + NCH:L + 2 * NCH]
        fgrid_sb = auxf[:, 160:161]
        cnt_sb = auxf[:, 161:162]
        seps_sb = auxf[:, 162:163]
        ones_sb = aux_sb[:, A_ONES:A_ONES + L]    # rows 0:2 used
        wvt_sb = aux_sb[:, A_WVT:A_WVT + 512]     # [128, 2*256]
        iden_sb = aux_sb[:, A_IDEN:A_IDEN + P]
        gstk_sb = cpool.tile([2, N], bf16, tag="gstk", name="gstk_sb")

        e_sb = cpool.tile([L, N], bf16, tag="e", name="e_sb")
        eg_sb = cpool.tile([L, N], bf16, tag="eg", name="eg_sb")
        xb0_sb = cpool.tile([L, C], bf16, tag="xb0", name="xb0_sb")
        xb1_sb = cpool.tile([L, C], bf16, tag="xb1", name="xb1_sb")
        x0t_sb = cpool.tile([P, 2, L], bf16, tag="x0t", name="x0t_sb")
        x1t_sb = cpool.tile([P, 2, L], bf16, tag="x1t", name="x1t_sb")
        hb0_sb = cpool.tile([L, OW], bf16, tag="hb0", name="hb0_sb")
        hb1_sb = cpool.tile([L, OW], bf16, tag="hb1", name="hb1_sb")

        # ---- input DMA. Transfers sharing a queue round-robin across its
        # DMA engines and complete together, so aux (which gates phase 1 +
        # masks) rides ALONE on sync, the earliest-starting queue. A tiny
        # ordering DMA then holds cut0's descriptors back until aux is
        # done, so cut0 still gets the sync queue to itself afterwards.
        # Cut sizes are matched to the measured queue start times
        # (sync ~8.7us, scalar ~10.2us, gpsimd ~12.4us).
        dummy_sb = cpool.tile([1, 2], bf16, tag="dummy", name="dummy_sb")
        nc.sync.dma_start(aux_sb[:], aux_d[:, :])
        nc.sync.dma_start(gstk_sb[:], gstk_d[:, :])
        nc.sync.dma_start(dummy_sb[0:1, :], aux_sb[0:1, 0:2])
        nc.sync.dma_start(xt_sb[:, 0:8 * C], xt_d[:, 0:8 * C])
        nc.scalar.dma_start(xt_sb[:, 8 * C:20 * C], xt_d[:, 8 * C:20 * C])
        nc.gpsimd.dma_start(xt_sb[:, 20 * C:], xt_d[:, 20 * C:])

        # warm tile: no input deps -> PE ramps p-state during input DMA
        warm_sb = cpool.tile([P, 512], bf16, tag="warm", name="warm_sb")
        nc.gpsimd.memset(warm_sb[:], 0.0)

        bctx = ExitStack()
        pgp = bctx.enter_context(tc.tile_pool(name="pgp", bufs=3, space="PSUM"))
        psbp = bctx.enter_context(tc.tile_pool(name="psbp", bufs=1, space="PSUM"))
        mkp = bctx.enter_context(tc.tile_pool(name="mkp", bufs=2))
        psb0 = psbp.tile([L, C], f32, tag="psb0", name="psb0")
        psb1 = psbp.tile([L, C], f32, tag="psb1", name="psb1")

        def warm(i, k):
            # shares the pg pool/tag so warm-up costs no extra PSUM bank
            pw = pgp.tile([L, 512], f32, tag="pg", name=f"pw{i}")
            for j in range(k):
                nc.tensor.matmul(pw[:], warm_sb[:, 0:L], warm_sb[:],
                                 start=(j == 0), stop=(j == k - 1))

        warm(0, 3)

        def xtc(n):
            return xt_sb[:, n * C:(n + 1) * C]

        pg_t = {}

        def bcast_exp(q):
            # pg[lev, 512q:512q+512] = g (exact: gh+gl sum in PSUM)
            lo = q * 512
            pg = pgp.tile([L, 512], f32, tag="pg", name=f"pg{q}")
            pg_t[q] = pg
            nc.tensor.matmul(pg[:], ones_sb[0:2, :],
                             gstk_sb[0:2, lo:lo + 512], start=True, stop=True)
            nc.scalar.activation(e_sb[:, lo:lo + 512], pg[:], Exp,
                                 scale=fgrid_sb[0:L, :])

        def eg4(q):
            # Eg = E * g; emitted after every is_eq so the in-order DVE
            # queue never stalls a mask the buckets are waiting on
            lo = q * 512
            pg = pg_t.pop(q)
            nc.vector.tensor_tensor(eg_sb[:, lo:lo + 512], e_sb[:, lo:lo + 512],
                                    pg[:], Alu.mult)

        def masks4(q):
            # one-hot masks for 4 chunks: mkb[p, j, lev] = (iota == idx)
            mkb = mkp.tile([P, 4, L], bf16, tag="mkb", name=f"mkb{q}")
            meb = mkp.tile([P, 4, L], bf16, tag="meb", name=f"meb{q}")
            iota3 = iota_sb.unsqueeze(1).broadcast_to([P, 4, L])
            idx3 = idx_sb[:, 4 * q:4 * q + 4].unsqueeze(2).broadcast_to(
                [P, 4, L])
            eps3 = eps_sb[:, 4 * q:4 * q + 4].unsqueeze(2).broadcast_to(
                [P, 4, L])
            nc.vector.tensor_tensor(mkb[:, :, :], iota3, idx3, Alu.is_equal)
            nc.gpsimd.tensor_mul(meb[:, :, :], mkb[:, :, :], eps3)
            return mkb, meb

        def buckets4(q, mkb, meb):
            for n in range(4 * q, 4 * q + 4):
                j = n % 4
                nc.tensor.matmul(psb0[:], mkb[:, j, :], xtc(n),
                                 start=(n == 0), stop=(n == NCH - 1))
                nc.tensor.matmul(psb1[:], meb[:, j, :], xtc(n),
                                 start=(n == 0), stop=(n == NCH - 1))

        # cnt/Seps columns of the H blocks come straight from the host
        nc.vector.tensor_copy(hb0_sb[:, C:C + 1], cnt_sb[0:L, :])
        nc.vector.tensor_copy(hb1_sb[:, C:C + 1], seps_sb[0:L, :])

        # All is_eq masks queue on DVE before any eg. Buckets are the
        # early phase (gated only by aux + xt cuts); bcast/exp/eg are
        # needed only by the main phase and slot between bucket groups
        # (pgp bufs=3 -> bcast q+3 needs eg(q), placed just ahead of it).
        # Warm matmuls fill the DMA-wait stalls so the PE p-state stays
        # ramped through the bucket phase.
        # all is_eq masks queue on DVE before any eg so no mask (which the
        # buckets wait on) gets stuck behind an exp-gated eg
        mks = [masks4(q) for q in range(8)]
        for q in range(8):
            bcast_exp(q)
            eg4(q)
            buckets4(q, *mks[q])
            if q == 6:
                # fill the tail DMA-wait so the PE p-state is ramped for
                # the last bucket group + transform + main phases
                warm(2, 6)

        # ---- transform: H = wv @ X (via TensorE transposes of X)
        ptp = bctx.enter_context(tc.tile_pool(name="ptp", bufs=1, space="PSUM"))
        phbp = bctx.enter_context(tc.tile_pool(name="phbp", bufs=1,
                                               space="PSUM"))
        warm(1, 5)  # keep clocks hot through the copy-bound bubble

        # both transform chains interleaved so scalar/vector/PE pipeline
        nc.scalar.copy(xb0_sb[:, 0:P], psb0[:, 0:P])
        nc.vector.tensor_copy(xb0_sb[:, P:C], psb0[:, P:C])
        nc.scalar.copy(xb1_sb[:, 0:P], psb1[:, 0:P])
        nc.vector.tensor_copy(xb1_sb[:, P:C], psb1[:, P:C])
        for cc in range(2):
            for (i, (xb, xtt)) in enumerate(((xb0_sb, x0t_sb),
                                             (xb1_sb, x1t_sb))):
                pt = ptp.tile([P, L], bf16, tag="pt", name=f"pt{i}{cc}")
                nc.tensor.transpose(pt[:], xb[0:L, cc * P:(cc + 1) * P],
                                    iden_sb[0:L, 0:L])
                if cc == 0:
                    nc.scalar.copy(xtt[:, cc, :], pt[:])
                else:
                    nc.vector.tensor_copy(xtt[:, cc, :], pt[:])
        phb0 = phbp.tile([L, C], f32, tag="phb0", name="phb0")
        phb1 = phbp.tile([L, C], f32, tag="phb1", name="phb1")
        for cc in range(2):
            nc.tensor.matmul(phb0[:], x0t_sb[:, cc, :],
                             wvt_sb[:, cc * C:(cc + 1) * C],
                             start=(cc == 0), stop=(cc == 1))
        for cc in range(2):
            nc.tensor.matmul(phb1[:], x1t_sb[:, cc, :],
                             wvt_sb[:, cc * C:(cc + 1) * C],
                             start=(cc == 0), stop=(cc == 1))
        nc.scalar.copy(hb0_sb[:, 0:P], phb0[:, 0:P])
        nc.vector.tensor_copy(hb0_sb[:, P:C], phb0[:, P:C])
        nc.scalar.copy(hb1_sb[:, 0:P], phb1[:, 0:P])
        nc.vector.tensor_copy(hb1_sb[:, P:C], phb1[:, P:C])
        bctx.close()

        # ---- main: po = E^T @ [H0|cnt] + Eg^T @ [H1|Seps]; the hb1 term
        # lags so the first matmuls start as soon as hb0 lands.
        OBAT = 4
        LAG = 5
        with tc.tile_pool(name="pop", bufs=8, space="PSUM") as pop, \
             tc.tile_pool(name="otp", bufs=4) as otp:
            pos = {}
            ot = None
            for t in range(NCH + LAG):
                if t < NCH:
                    po = pop.tile([P, OW], f32, tag="po", name=f"po{t}")
                    pos[t] = po
                    nc.tensor.matmul(po[:], e_sb[:, t * P:(t + 1) * P],
                                     hb0_sb[:], start=True, stop=False)
                if t >= LAG:
                    mc = t - LAG
                    po2 = pos.pop(mc)
                    nc.tensor.matmul(po2[:], eg_sb[:, mc * P:(mc + 1) * P],
                                     hb1_sb[:], start=False, stop=True)
                    k = mc % OBAT
                    if k == 0:
                        ot = otp.tile([P, OBAT * OW], bf16, tag="ot",
                                      name=f"ot{mc // OBAT}")
                    dst = ot[:, k * OW:(k + 1) * OW]
                    if mc % 2 == 0:
                        nc.scalar.copy(dst, po2[:])
                    else:
                        nc.vector.tensor_copy(dst, po2[:])
                    if k == OBAT - 1:
                        ob = mc // OBAT
                        c0 = ob * OBAT * OW
                        oq = nc.sync if ob % 2 == 0 else nc.gpsimd
                        oq.dma_start(o_d[:, c0:c0 + OBAT * OW], ot[:])

    nc.compile()
    return nc


def _get_nc():
    if "nc" not in _CACHE:
        _CACHE["nc"] = _build()
    return _CACHE["nc"]


def make_in_maps(x, wq, wk, wv):
    import ml_dtypes
    bf = ml_dtypes.bfloat16
    xf = np.ascontiguousarray(x, dtype=np.float32).reshape(B, C, N)
    wq = np.asarray(wq, dtype=np.float32).reshape(C)
    wk = np.asarray(wk, dtype=np.float32).reshape(C)
    wv = np.asarray(wv, dtype=np.float32)

    fgrid = np.arange(L, dtype=np.float32) * DELTA - FRNG

    in_maps = []
    for b in range(B):
        xb = xf[b]                                   # [C, N]
        f = wq @ xb                                  # [N] exact fp32
        g = wk @ xb
        idx = np.clip(np.round((f + FRNG) / DELTA), 0, L - 1).astype(
            np.int64)
        eps = f - fgrid[idx]
        cnt = np.bincount(idx, minlength=L).astype(np.float32)
        seps = np.bincount(idx, weights=eps.astype(np.float64),
                           minlength=L).astype(np.float32)

        xt = np.ascontiguousarray(
            xb.T.reshape(NCH, P, C).transpose(1, 0, 2).reshape(P, XTW)
        ).astype(bf)

        gh = g.astype(bf)
        gl = (g - gh.astype(np.float32)).astype(bf)
        gstk = np.stack([gh, gl])                     # [2, N]

        aux = np.zeros((P, AUXW), dtype=bf)
        auxf = aux[:, 0:512].view(np.float32)
        auxf[:, 0:L] = np.arange(L, dtype=np.float32)[None, :]
        auxf[:, L:L + NCH] = idx.astype(np.float32).reshape(NCH, P).T
        auxf[:, L + NCH:L + 2 * NCH] = eps.reshape(NCH, P).T
        auxf[0:L, 160] = fgrid
        auxf[0:L, 161] = cnt
        auxf[0:L, 162] = seps
        aux[0:2, A_ONES:A_ONES + L] = bf(1.0)
        aux[:, A_WVT:A_WVT + C] = wv[:, 0:P].T.astype(bf)
        aux[:, A_WVT + C:A_WVT + 2 * C] = wv[:, P:2 * P].T.astype(bf)
        aux[:, A_IDEN:A_IDEN + P] = np.eye(P, dtype=np.float32).astype(bf)

        in_maps.append({"xt": xt, "aux": aux, "gstk": gstk})
    return in_maps, xf


def kernel(x, wq, wk, wv, gamma):
    from concourse.bass_utils import run_bass_kernel_spmd

    in_maps, xf = make_in_maps(x, wq, wk, wv)
    nc = _get_nc()
    res = run_bass_kernel_spmd(nc, in_maps, core_ids=list(range(B)))

    g0 = float(np.asarray(gamma, dtype=np.float32).reshape(-1)[0])
    out = np.empty((B, C, HH, WW), dtype=np.float32)
    for b in range(B):
        onz = res.results[b]["o"].astype(np.float32)  # [P, NCH*257]
        onz = onz.reshape(P, NCH, OW).transpose(1, 0, 2).reshape(N, OW)
        o = (onz[:, 0:C] / onz[:, C:C + 1]).T         # [C, N]
        if g0 != 0.0:
            o = (1.0 - g0) * o + g0 * xf[b]
        out[b] = o.reshape(C, HH, WW)
    return out


# revision 47
# speedup vs baseline: 1.1065x; 1.0777x over previous
"""Trainium2 Bass kernel for the AttentionHook module.

Math (per batch b, N = H*W = 4096):
    f = wq @ x   [N];   g = wk @ x   [N];   h = wv @ x   [C, N]
    scores[i, j] = f[i] * g[j]      (rank-1 outer product!)
    beta = softmax(scores, axis=0)  (normalize over i, per column j)
    o = (1-gamma) * h @ beta + gamma * x
Because scores are rank-1, quantize f onto a uniform grid of L=96 levels
(f = fhat + eps) and bucket by level; first-order eps correction:
    sum_n h[c,n] e^{f_n g_m} ~= sum_lev e^{fhat_lev g_m} (H0 + g_m H1)[c,lev]

Key structural choices vs the v0 kernel:
  * bucket RAW x (shipped n-major as x^T chunks straight from DMA) and
    apply wv AFTER bucketing: H0 = wv @ X0 — turns the [C, N] h-compute
    plus its PSUM->SBUF staging into a tiny [C, L] transform.
  * f, g, idx, eps, cnt, Seps are exact fp32 on the host (2 matvecs +
    bincounts, the same scale of host work as the final divide). g is
    re-broadcast on-device across the 96 level partitions with
    16-partition selector matmuls against a bf16 [gh; gl] stack so the
    hi+lo split of g sums exactly in PSUM.

Per core (one batch per core, 8 cores):
  bcast: pg[lev, m] = g_m (f32-accurate) via sel^T @ [gh; gl] matmuls.
  exp:   E = exp(pg * fgrid) on ScalarE; Eg = E * pg on DVE.
  masks: one-hot (iota == idx) on DVE; meb = mask*eps on GpSimd.
  bucket: X0/X1 += mask^T @ x^T per 128-chunk (TensorE).
  transform: Xk -> SBUF bf16, 2 TensorE transposes each, Hk = Xk^T @ wv^T.
  main:  po[m, 0:257] = E^T @ [H0|cnt] + Eg^T @ [H1|Seps] (PSUM accum).
  out:   ship [num | Z] bf16 chunk-major; host divides + transposes.
"""

import numpy as np
from contextlib import ExitStack

B, C, HH, WW = 8, 256, 64, 64
N = HH * WW            # 4096
P = 128
NCH = N // P           # 32 n-chunks (also m-chunks)
L = 96                 # f-quantization levels
FRNG = 6.0             # f grid covers [-6, 6)
DELTA = 2 * FRNG / L   # 0.125
OW = C + 1             # output row width: [num(256) | Z]
XTW = NCH * C          # 8192 bf16 cols in the x^T blob

# aux blob (bf16 cols; f32 regions live in the first 512 bf16 cols)
#   f32 view cols: iota 0:96 | idx 96:128 | eps 128:160 | fgrid 160 |
#                  cnt 161 | Seps 162
A_GSTK = 512           # gstk [16, 512] bf16 (gh/gl interleaved rows)
A_SEL = 1024           # selectors [16, 8*96]: rows 2q,2q+1 of block q = 1
A_WVT = 1792           # wv^T c-chunks [128, 2*256]
A_IDEN = 2304          # identity [128, 128]
AUXW = 2432

_CACHE = {}


def _build():
    import concourse.tile as tile
    from concourse import bacc, mybir

    f32 = mybir.dt.float32
    bf16 = mybir.dt.bfloat16
    Exp = mybir.ActivationFunctionType.Exp
    Alu = mybir.AluOpType

    nc = bacc.Bacc("TRN2", target_bir_lowering=False, debug=False)
    xt_d = nc.dram_tensor("xt", [P, XTW], bf16, kind="ExternalInput").ap()
    aux_d = nc.dram_tensor("aux", [P, AUXW], bf16, kind="ExternalInput").ap()
    o_d = nc.dram_tensor("o", [P, NCH * OW], bf16, kind="ExternalOutput").ap()

    with tile.TileContext(nc) as tc, ExitStack() as ctx:
        cpool = ctx.enter_context(tc.tile_pool(name="cpool", bufs=1))
        xt_sb = cpool.tile([P, XTW], bf16, tag="xt", name="xt_sb")
        aux_sb = cpool.tile([P, AUXW], bf16, tag="aux", name="aux_sb")
        auxf = aux_sb[:, 0:512].bitcast(f32)      # [128, 256] f32 view
        iota_sb = auxf[:, 0:L]
        idx_sb = auxf[:, L:L + NCH]
        eps_sb = auxf[:, L + NCH:L + 2 * NCH]
        fgrid_sb = auxf[:, 160:161]
        cnt_sb = auxf[:, 161:162]
        seps_sb = auxf[:, 162:163]
        gstk_sb = aux_sb[:, A_GSTK:A_GSTK + 512]  # rows 0:16 used
        sel_sb = aux_sb[:, A_SEL:A_SEL + 8 * L]   # rows 0:16 used
        wvt_sb = aux_sb[:, A_WVT:A_WVT + 512]     # [128, 2*256]
        iden_sb = aux_sb[:, A_IDEN:A_IDEN + P]

        e_sb = cpool.tile([L, N], bf16, tag="e", name="e_sb")
        eg_sb = cpool.tile([L, N], bf16, tag="eg", name="eg_sb")
        xb0_sb = cpool.tile([L, C], bf16, tag="xb0", name="xb0_sb")
        xb1_sb = cpool.tile([L, C], bf16, tag="xb1", name="xb1_sb")
        x0t_sb = cpool.tile([P, 2, L], bf16, tag="x0t", name="x0t_sb")
        x1t_sb = cpool.tile([P, 2, L], bf16, tag="x1t", name="x1t_sb")
        hb0_sb = cpool.tile([L, OW], bf16, tag="hb0", name="hb0_sb")
        hb1_sb = cpool.tile([L, OW], bf16, tag="hb1", name="hb1_sb")

        # ---- input DMA: aux first on the scalar queue, x^T in 4 cuts
        nc.scalar.dma_start(aux_sb[:], aux_d[:, :])
        CUT = XTW // 4
        qin = [nc.sync, nc.gpsimd, nc.gpsimd, nc.sync]
        for k in range(4):
            qin[k].dma_start(xt_sb[:, k * CUT:(k + 1) * CUT],
                             xt_d[:, k * CUT:(k + 1) * CUT])

        bctx = ExitStack()
        pgp = bctx.enter_context(tc.tile_pool(name="pgp", bufs=2, space="PSUM"))
        psbp = bctx.enter_context(tc.tile_pool(name="psbp", bufs=1, space="PSUM"))
        mkp = bctx.enter_context(tc.tile_pool(name="mkp", bufs=2))
        psb0 = psbp.tile([L, C], f32, tag="psb0", name="psb0")
        psb1 = psbp.tile([L, C], f32, tag="psb1", name="psb1")

        def xtc(n):
            return xt_sb[:, n * C:(n + 1) * C]

        def bcast_exp(q):
            # pg[lev, 512q:512q+512] = g (exact: gh+gl sum in PSUM)
            lo = q * 512
            pg = pgp.tile([L, 512], f32, tag="pg", name=f"pg{q}")
            nc.tensor.matmul(pg[:], sel_sb[0:16, q * L:(q + 1) * L],
                             gstk_sb[0:16, :], start=True, stop=True)
            nc.scalar.activation(e_sb[:, lo:lo + 512], pg[:], Exp,
                                 scale=fgrid_sb[0:L, :])
            nc.vector.tensor_tensor(eg_sb[:, lo:lo + 512], e_sb[:, lo:lo + 512],
                                    pg[:], Alu.mult)

        def masks4(q):
            # one-hot masks for 4 chunks: mkb[p, j, lev] = (iota == idx)
            mkb = mkp.tile([P, 4, L], bf16, tag="mkb", name=f"mkb{q}")
            meb = mkp.tile([P, 4, L], bf16, tag="meb", name=f"meb{q}")
            iota3 = iota_sb.unsqueeze(1).broadcast_to([P, 4, L])
            idx3 = idx_sb[:, 4 * q:4 * q + 4].unsqueeze(2).broadcast_to(
                [P, 4, L])
            eps3 = eps_sb[:, 4 * q:4 * q + 4].unsqueeze(2).broadcast_to(
                [P, 4, L])
            nc.vector.tensor_tensor(mkb[:, :, :], iota3, idx3, Alu.is_equal)
            nc.gpsimd.tensor_mul(meb[:, :, :], mkb[:, :, :], eps3)
            return mkb, meb

        def buckets4(q, mkb, meb):
            for n in range(4 * q, 4 * q + 4):
                j = n % 4
                nc.tensor.matmul(psb0[:], mkb[:, j, :], xtc(n),
                                 start=(n == 0), stop=(n == NCH - 1))
                nc.tensor.matmul(psb1[:], meb[:, j, :], xtc(n),
                                 start=(n == 0), stop=(n == NCH - 1))

        # cnt/Seps columns of the H blocks come straight from the host
        nc.vector.tensor_copy(hb0_sb[:, C:C + 1], cnt_sb[0:L, :])
        nc.vector.tensor_copy(hb1_sb[:, C:C + 1], seps_sb[0:L, :])

        for q in range(8):
            bcast_exp(q)
            mkb, meb = masks4(q)
            buckets4(q, mkb, meb)

        # ---- transform: H = wv @ X (via TensorE transposes of X);
        # both chains interleaved so scalar/vector/PE pipeline
        ptp = bctx.enter_context(tc.tile_pool(name="ptp", bufs=2, space="PSUM"))
        phbp = bctx.enter_context(tc.tile_pool(name="phbp", bufs=1,
                                               space="PSUM"))
        nc.scalar.copy(xb0_sb[:, 0:P], psb0[:, 0:P])
        nc.vector.tensor_copy(xb0_sb[:, P:C], psb0[:, P:C])
        nc.scalar.copy(xb1_sb[:, 0:P], psb1[:, 0:P])
        nc.vector.tensor_copy(xb1_sb[:, P:C], psb1[:, P:C])
        for cc in range(2):
            for (i, (xb, xtt)) in enumerate(((xb0_sb, x0t_sb),
                                             (xb1_sb, x1t_sb))):
                pt = ptp.tile([P, L], bf16, tag="pt", name=f"pt{i}{cc}")
                nc.tensor.transpose(pt[:], xb[0:L, cc * P:(cc + 1) * P],
                                    iden_sb[0:L, 0:L])
                if cc == 0:
                    nc.scalar.copy(xtt[:, cc, :], pt[:])
                else:
                    nc.vector.tensor_copy(xtt[:, cc, :], pt[:])
        phb0 = phbp.tile([L, C], f32, tag="phb0", name="phb0")
        phb1 = phbp.tile([L, C], f32, tag="phb1", name="phb1")
        for cc in range(2):
            nc.tensor.matmul(phb0[:], x0t_sb[:, cc, :],
                             wvt_sb[:, cc * C:(cc + 1) * C],
                             start=(cc == 0), stop=(cc == 1))
        for cc in range(2):
            nc.tensor.matmul(phb1[:], x1t_sb[:, cc, :],
                             wvt_sb[:, cc * C:(cc + 1) * C],
                             start=(cc == 0), stop=(cc == 1))
        nc.scalar.copy(hb0_sb[:, 0:P], phb0[:, 0:P])
        nc.vector.tensor_copy(hb0_sb[:, P:C], phb0[:, P:C])
        nc.scalar.copy(hb1_sb[:, 0:P], phb1[:, 0:P])
        nc.vector.tensor_copy(hb1_sb[:, P:C], phb1[:, P:C])
        bctx.close()

        # ---- main: po = E^T @ [H0|cnt] + Eg^T @ [H1|Seps]; batched out DMA
        OBAT = 4
        with tc.tile_pool(name="pop", bufs=8, space="PSUM") as pop, \
             tc.tile_pool(name="otp", bufs=4) as otp:
            for ob in range(NCH // OBAT):
                ot = otp.tile([P, OBAT * OW], bf16, tag="ot", name=f"ot{ob}")
                for k in range(OBAT):
                    mc = ob * OBAT + k
                    po = pop.tile([P, OW], f32, tag="po", name=f"po{mc}")
                    nc.tensor.matmul(po[:], e_sb[:, mc * P:(mc + 1) * P],
                                     hb0_sb[:], start=True, stop=False)
                    nc.tensor.matmul(po[:], eg_sb[:, mc * P:(mc + 1) * P],
                                     hb1_sb[:], start=False, stop=True)
                    dst = ot[:, k * OW:(k + 1) * OW]
                    if mc % 2 == 0:
                        nc.scalar.copy(dst, po[:])
                    else:
                        nc.vector.tensor_copy(dst, po[:])
                c0 = ob * OBAT * OW
                oq = nc.sync if ob % 2 == 0 else nc.gpsimd
                oq.dma_start(o_d[:, c0:c0 + OBAT * OW], ot[:])

    nc.compile()
    return nc


def _get_nc():
    if "nc" not in _CACHE:
        _CACHE["nc"] = _build()
    return _CACHE["nc"]


def make_in_maps(x, wq, wk, wv):
    import ml_dtypes
    bf = ml_dtypes.bfloat16
    xf = np.ascontiguousarray(x, dtype=np.float32).reshape(B, C, N)
    wq = np.asarray(wq, dtype=np.float32).reshape(C)
    wk = np.asarray(wk, dtype=np.float32).reshape(C)
    wv = np.asarray(wv, dtype=np.float32)

    fgrid = np.arange(L, dtype=np.float32) * DELTA - FRNG

    in_maps = []
    for b in range(B):
        xb = xf[b]                                   # [C, N]
        f = wq @ xb                                  # [N] exact fp32
        g = wk @ xb
        idx = np.clip(np.round((f + FRNG) / DELTA), 0, L - 1).astype(
            np.int64)
        eps = f - fgrid[idx]
        cnt = np.bincount(idx, minlength=L).astype(np.float32)
        seps = np.bincount(idx, weights=eps.astype(np.float64),
                           minlength=L).astype(np.float32)

        xt = np.ascontiguousarray(
            xb.T.reshape(NCH, P, C).transpose(1, 0, 2).reshape(P, XTW)
        ).astype(bf)

        gh = g.astype(bf)
        gl = (g - gh.astype(np.float32)).astype(bf)
        gstk = np.zeros((P, 512), dtype=bf)
        for q in range(8):
            gstk[2 * q] = gh.reshape(8, 512)[q]
            gstk[2 * q + 1] = gl.reshape(8, 512)[q]

        aux = np.zeros((P, AUXW), dtype=bf)
        auxf = aux[:, 0:512].view(np.float32)
        auxf[:, 0:L] = np.arange(L, dtype=np.float32)[None, :]
        auxf[:, L:L + NCH] = idx.astype(np.float32).reshape(NCH, P).T
        auxf[:, L + NCH:L + 2 * NCH] = eps.reshape(NCH, P).T
        auxf[0:L, 160] = fgrid
        auxf[0:L, 161] = cnt
        auxf[0:L, 162] = seps
        aux[:, A_GSTK:A_GSTK + 512] = gstk
        for q in range(8):
            aux[2 * q:2 * q + 2, A_SEL + q * L:A_SEL + (q + 1) * L] = bf(1.0)
        aux[:, A_WVT:A_WVT + C] = wv[:, 0:P].T.astype(bf)
        aux[:, A_WVT + C:A_WVT + 2 * C] = wv[:, P:2 * P].T.astype(bf)
        aux[:, A_IDEN:A_IDEN + P] = np.eye(P, dtype=np.float32).astype(bf)

        in_maps.append({"xt": xt, "aux": aux})
    return in_maps, xf


def kernel(x, wq, wk, wv, gamma):
    from concourse.bass_utils import run_bass_kernel_spmd

    in_maps, xf = make_in_maps(x, wq, wk, wv)
    nc = _get_nc()
    res = run_bass_kernel_spmd(nc, in_maps, core_ids=list(range(B)))

    g0 = float(np.asarray(gamma, dtype=np.float32).reshape(-1)[0])
    out = np.empty((B, C, HH, WW), dtype=np.float32)
    for b in range(B):
        onz = res.results[b]["o"].astype(np.float32)  # [P, NCH*257]
        onz = onz.reshape(P, NCH, OW).transpose(1, 0, 2).reshape(N, OW)
        o = (onz[:, 0:C] / onz[:, C:C + 1]).T         # [C, N]
        if g0 != 0.0:
            o = (1.0 - g0) * o + g0 * xf[b]
        out[b] = o.reshape(C, HH, WW)
    return out


# revision 48
# speedup vs baseline: 1.2024x; 1.0866x over previous
"""Trainium2 Bass kernel for the AttentionHook module.

Math (per batch b, N = H*W = 4096):
    f = wq @ x   [N];   g = wk @ x   [N];   h = wv @ x   [C, N]
    scores[i, j] = f[i] * g[j]      (rank-1 outer product!)
    beta = softmax(scores, axis=0)  (normalize over i, per column j)
    o = (1-gamma) * h @ beta + gamma * x
Because scores are rank-1, quantize f onto a uniform grid of L=96 levels
(f = fhat + eps) and bucket by level; first-order eps correction:
    sum_n h[c,n] e^{f_n g_m} ~= sum_lev e^{fhat_lev g_m} (H0 + g_m H1)[c,lev]

Key structural choices vs the v0 kernel:
  * bucket RAW x (shipped n-major as x^T chunks straight from DMA) and
    apply wv AFTER bucketing: H0 = wv @ X0 — turns the [C, N] h-compute
    plus its PSUM->SBUF staging into a tiny [C, L] transform.
  * f, g, idx, eps, cnt, Seps are exact fp32 on the host (2 matvecs +
    bincounts, the same scale of host work as the final divide). g is
    re-broadcast on-device across the 96 level partitions with
    16-partition selector matmuls against a bf16 [gh; gl] stack so the
    hi+lo split of g sums exactly in PSUM.

Per core (one batch per core, 8 cores):
  bcast: pg[lev, m] = g_m (f32-accurate) via sel^T @ [gh; gl] matmuls.
  exp:   E = exp(pg * fgrid) on ScalarE; Eg = E * pg on DVE.
  masks: one-hot (iota == idx) on DVE; meb = mask*eps on GpSimd.
  bucket: X0/X1 += mask^T @ x^T per 128-chunk (TensorE).
  transform: Xk -> SBUF bf16, 2 TensorE transposes each, Hk = Xk^T @ wv^T.
  main:  po[m, 0:257] = E^T @ [H0|cnt] + Eg^T @ [H1|Seps] (PSUM accum).
  out:   ship [num | Z] bf16 chunk-major; host divides + transposes.
"""

import numpy as np
from contextlib import ExitStack

B, C, HH, WW = 8, 256, 64, 64
N = HH * WW            # 4096
P = 128
NCH = N // P           # 32 n-chunks (also m-chunks)
L = 96                 # f-quantization levels
FRNG = 6.0             # f grid covers [-6, 6)
DELTA = 2 * FRNG / L   # 0.125
OW = C + 1             # output row width: [num(256) | Z]
XTW = NCH * C          # 8192 bf16 cols in the x^T blob

# aux blob (bf16 cols; f32 regions live in the first 512 bf16 cols)
#   f32 view cols: iota 0:96 | idx 96:128 | eps 128:160 | fgrid 160 |
#                  cnt 161 | Seps 162
A_GSTK = 512           # gstk [16, 512] bf16 (gh/gl interleaved rows)
A_SEL = 1024           # selectors [16, 8*96]: rows 2q,2q+1 of block q = 1
A_WVT = 1792           # wv^T c-chunks [128, 2*256]
A_IDEN = 2304          # identity [128, 128]
AUXW = 2432

_CACHE = {}


def _build():
    import concourse.tile as tile
    from concourse import bacc, mybir

    f32 = mybir.dt.float32
    bf16 = mybir.dt.bfloat16
    Exp = mybir.ActivationFunctionType.Exp
    Alu = mybir.AluOpType

    nc = bacc.Bacc("TRN2", target_bir_lowering=False, debug=False)
    xt_d = nc.dram_tensor("xt", [P, XTW], bf16, kind="ExternalInput").ap()
    aux_d = nc.dram_tensor("aux", [P, AUXW], bf16, kind="ExternalInput").ap()
    o_d = nc.dram_tensor("o", [P, NCH * OW], bf16, kind="ExternalOutput").ap()

    with tile.TileContext(nc) as tc, ExitStack() as ctx:
        cpool = ctx.enter_context(tc.tile_pool(name="cpool", bufs=1))
        xt_sb = cpool.tile([P, XTW], bf16, tag="xt", name="xt_sb")
        aux_sb = cpool.tile([P, AUXW], bf16, tag="aux", name="aux_sb")
        auxf = aux_sb[:, 0:512].bitcast(f32)      # [128, 256] f32 view
        iota_sb = auxf[:, 0:L]
        idx_sb = auxf[:, L:L + NCH]
        eps_sb = auxf[:, L + NCH:L + 2 * NCH]
        fgrid_sb = auxf[:, 160:161]
        cnt_sb = auxf[:, 161:162]
        seps_sb = auxf[:, 162:163]
        gstk_sb = aux_sb[:, A_GSTK:A_GSTK + 512]  # rows 0:16 used
        sel_sb = aux_sb[:, A_SEL:A_SEL + 8 * L]   # rows 0:16 used
        wvt_sb = aux_sb[:, A_WVT:A_WVT + 512]     # [128, 2*256]
        iden_sb = aux_sb[:, A_IDEN:A_IDEN + P]

        e_sb = cpool.tile([L, N], bf16, tag="e", name="e_sb")
        eg_sb = cpool.tile([L, N], bf16, tag="eg", name="eg_sb")
        xb0_sb = cpool.tile([L, C], bf16, tag="xb0", name="xb0_sb")
        xb1_sb = cpool.tile([L, C], bf16, tag="xb1", name="xb1_sb")
        x0t_sb = cpool.tile([P, 2, L], bf16, tag="x0t", name="x0t_sb")
        x1t_sb = cpool.tile([P, 2, L], bf16, tag="x1t", name="x1t_sb")
        hb0_sb = cpool.tile([L, OW], bf16, tag="hb0", name="hb0_sb")
        hb1_sb = cpool.tile([L, OW], bf16, tag="hb1", name="hb1_sb")

        # ---- input DMA. Compute running during the input stream throttles
        # it (SBUF port contention), so aux — which gates all compute — is
        # deliberately co-queued to finish WITH the x^T stream, not before.
        CUT = XTW // 4
        nc.sync.dma_start(xt_sb[:, 0:CUT], xt_d[:, 0:CUT])
        nc.sync.dma_start(xt_sb[:, 3 * CUT:], xt_d[:, 3 * CUT:])
        nc.scalar.dma_start(xt_sb[:, CUT:2 * CUT], xt_d[:, CUT:2 * CUT])
        nc.scalar.dma_start(aux_sb[:], aux_d[:, :])
        nc.gpsimd.dma_start(xt_sb[:, 2 * CUT:3 * CUT],
                            xt_d[:, 2 * CUT:3 * CUT])

        bctx = ExitStack()
        pgp = bctx.enter_context(tc.tile_pool(name="pgp", bufs=2, space="PSUM"))
        psbp = bctx.enter_context(tc.tile_pool(name="psbp", bufs=1, space="PSUM"))
        mkp = bctx.enter_context(tc.tile_pool(name="mkp", bufs=2))
        psb0 = psbp.tile([L, C], f32, tag="psb0", name="psb0")
        psb1 = psbp.tile([L, C], f32, tag="psb1", name="psb1")

        def xtc(n):
            return xt_sb[:, n * C:(n + 1) * C]

        def bcast_exp(q):
            # pg[lev, 512q:512q+512] = g (exact: gh+gl sum in PSUM)
            lo = q * 512
            pg = pgp.tile([L, 512], f32, tag="pg", name=f"pg{q}")
            nc.tensor.matmul(pg[:], sel_sb[0:16, q * L:(q + 1) * L],
                             gstk_sb[0:16, :], start=True, stop=True)
            nc.scalar.activation(e_sb[:, lo:lo + 512], pg[:], Exp,
                                 scale=fgrid_sb[0:L, :])
            nc.vector.tensor_tensor(eg_sb[:, lo:lo + 512], e_sb[:, lo:lo + 512],
                                    pg[:], Alu.mult)

        def masks4(q):
            # one-hot masks for 4 chunks: mkb[p, j, lev] = (iota == idx)
            mkb = mkp.tile([P, 4, L], bf16, tag="mkb", name=f"mkb{q}")
            meb = mkp.tile([P, 4, L], bf16, tag="meb", name=f"meb{q}")
            iota3 = iota_sb.unsqueeze(1).broadcast_to([P, 4, L])
            idx3 = idx_sb[:, 4 * q:4 * q + 4].unsqueeze(2).broadcast_to(
                [P, 4, L])
            eps3 = eps_sb[:, 4 * q:4 * q + 4].unsqueeze(2).broadcast_to(
                [P, 4, L])
            nc.vector.tensor_tensor(mkb[:, :, :], iota3, idx3, Alu.is_equal)
            nc.gpsimd.tensor_mul(meb[:, :, :], mkb[:, :, :], eps3)
            return mkb, meb

        def buckets4(q, mkb, meb):
            for n in range(4 * q, 4 * q + 4):
                j = n % 4
                nc.tensor.matmul(psb0[:], mkb[:, j, :], xtc(n),
                                 start=(n == 0), stop=(n == NCH - 1))
                nc.tensor.matmul(psb1[:], meb[:, j, :], xtc(n),
                                 start=(n == 0), stop=(n == NCH - 1))

        # cnt/Seps columns of the H blocks come straight from the host
        nc.vector.tensor_copy(hb0_sb[:, C:C + 1], cnt_sb[0:L, :])
        nc.vector.tensor_copy(hb1_sb[:, C:C + 1], seps_sb[0:L, :])

        for q in range(8):
            bcast_exp(q)
            mkb, meb = masks4(q)
            buckets4(q, mkb, meb)

        # ---- transform: H = wv @ X (via TensorE transposes of X);
        # both chains interleaved so scalar/vector/PE pipeline
        ptp = bctx.enter_context(tc.tile_pool(name="ptp", bufs=2, space="PSUM"))
        phbp = bctx.enter_context(tc.tile_pool(name="phbp", bufs=1,
                                               space="PSUM"))
        nc.scalar.copy(xb0_sb[:, 0:P], psb0[:, 0:P])
        nc.vector.tensor_copy(xb0_sb[:, P:C], psb0[:, P:C])
        nc.scalar.copy(xb1_sb[:, 0:P], psb1[:, 0:P])
        nc.vector.tensor_copy(xb1_sb[:, P:C], psb1[:, P:C])
        for cc in range(2):
            for (i, (xb, xtt)) in enumerate(((xb0_sb, x0t_sb),
                                             (xb1_sb, x1t_sb))):
                pt = ptp.tile([P, L], bf16, tag="pt", name=f"pt{i}{cc}")
                nc.tensor.transpose(pt[:], xb[0:L, cc * P:(cc + 1) * P],
                                    iden_sb[0:L, 0:L])
                if cc == 0:
                    nc.scalar.copy(xtt[:, cc, :], pt[:])
                else:
                    nc.vector.tensor_copy(xtt[:, cc, :], pt[:])
        phb0 = phbp.tile([L, C], f32, tag="phb0", name="phb0")
        phb1 = phbp.tile([L, C], f32, tag="phb1", name="phb1")
        for cc in range(2):
            nc.tensor.matmul(phb0[:], x0t_sb[:, cc, :],
                             wvt_sb[:, cc * C:(cc + 1) * C],
                             start=(cc == 0), stop=(cc == 1))
        for cc in range(2):
            nc.tensor.matmul(phb1[:], x1t_sb[:, cc, :],
                             wvt_sb[:, cc * C:(cc + 1) * C],
                             start=(cc == 0), stop=(cc == 1))
        nc.scalar.copy(hb0_sb[:, 0:P], phb0[:, 0:P])
        nc.vector.tensor_copy(hb0_sb[:, P:C], phb0[:, P:C])
        nc.scalar.copy(hb1_sb[:, 0:P], phb1[:, 0:P])
        nc.vector.tensor_copy(hb1_sb[:, P:C], phb1[:, P:C])
        bctx.close()

        # ---- main: po = E^T @ [H0|cnt] + Eg^T @ [H1|Seps]; batched out DMA
        OBAT = 4
        with tc.tile_pool(name="pop", bufs=8, space="PSUM") as pop, \
             tc.tile_pool(name="otp", bufs=4) as otp:
            for ob in range(NCH // OBAT):
                ot = otp.tile([P, OBAT * OW], bf16, tag="ot", name=f"ot{ob}")
                for k in range(OBAT):
                    mc = ob * OBAT + k
                    po = pop.tile([P, OW], f32, tag="po", name=f"po{mc}")
                    nc.tensor.matmul(po[:], e_sb[:, mc * P:(mc + 1) * P],
                                     hb0_sb[:], start=True, stop=False)
                    nc.tensor.matmul(po[:], eg_sb[:, mc * P:(mc + 1) * P],
                                     hb1_sb[:], start=False, stop=True)
                    dst = ot[:, k * OW:(k + 1) * OW]
                    if mc % 2 == 0:
                        nc.scalar.copy(dst, po[:])
                    else:
                        nc.vector.tensor_copy(dst, po[:])
                c0 = ob * OBAT * OW
                oq = nc.sync if ob % 2 == 0 else nc.gpsimd
                oq.dma_start(o_d[:, c0:c0 + OBAT * OW], ot[:])

    nc.compile()
    return nc


def _get_nc():
    if "nc" not in _CACHE:
        _CACHE["nc"] = _build()
    return _CACHE["nc"]


def make_in_maps(x, wq, wk, wv):
    import ml_dtypes
    bf = ml_dtypes.bfloat16
    xf = np.ascontiguousarray(x, dtype=np.float32).reshape(B, C, N)
    wq = np.asarray(wq, dtype=np.float32).reshape(C)
    wk = np.asarray(wk, dtype=np.float32).reshape(C)
    wv = np.asarray(wv, dtype=np.float32)

    fgrid = np.arange(L, dtype=np.float32) * DELTA - FRNG

    in_maps = []
    for b in range(B):
        xb = xf[b]                                   # [C, N]
        f = wq @ xb                                  # [N] exact fp32
        g = wk @ xb
        idx = np.clip(np.round((f + FRNG) / DELTA), 0, L - 1).astype(
            np.int64)
        eps = f - fgrid[idx]
        cnt = np.bincount(idx, minlength=L).astype(np.float32)
        seps = np.bincount(idx, weights=eps.astype(np.float64),
                           minlength=L).astype(np.float32)

        xt = np.ascontiguousarray(
            xb.T.reshape(NCH, P, C).transpose(1, 0, 2).reshape(P, XTW)
        ).astype(bf)

        gh = g.astype(bf)
        gl = (g - gh.astype(np.float32)).astype(bf)
        gstk = np.zeros((P, 512), dtype=bf)
        for q in range(8):
            gstk[2 * q] = gh.reshape(8, 512)[q]
            gstk[2 * q + 1] = gl.reshape(8, 512)[q]

        aux = np.zeros((P, AUXW), dtype=bf)
        auxf = aux[:, 0:512].view(np.float32)
        auxf[:, 0:L] = np.arange(L, dtype=np.float32)[None, :]
        auxf[:, L:L + NCH] = idx.astype(np.float32).reshape(NCH, P).T
        auxf[:, L + NCH:L + 2 * NCH] = eps.reshape(NCH, P).T
        auxf[0:L, 160] = fgrid
        auxf[0:L, 161] = cnt
        auxf[0:L, 162] = seps
        aux[:, A_GSTK:A_GSTK + 512] = gstk
        for q in range(8):
            aux[2 * q:2 * q + 2, A_SEL + q * L:A_SEL + (q + 1) * L] = bf(1.0)
        aux[:, A_WVT:A_WVT + C] = wv[:, 0:P].T.astype(bf)
        aux[:, A_WVT + C:A_WVT + 2 * C] = wv[:, P:2 * P].T.astype(bf)
        aux[:, A_IDEN:A_IDEN + P] = np.eye(P, dtype=np.float32).astype(bf)

        in_maps.append({"xt": xt, "aux": aux})
    return in_maps, xf


def kernel(x, wq, wk, wv, gamma):
    from concourse.bass_utils import run_bass_kernel_spmd

    in_maps, xf = make_in_maps(x, wq, wk, wv)
    nc = _get_nc()
    res = run_bass_kernel_spmd(nc, in_maps, core_ids=list(range(B)))

    g0 = float(np.asarray(gamma, dtype=np.float32).reshape(-1)[0])
    out = np.empty((B, C, HH, WW), dtype=np.float32)
    for b in range(B):
        onz = res.results[b]["o"].astype(np.float32)  # [P, NCH*257]
        onz = onz.reshape(P, NCH, OW).transpose(1, 0, 2).reshape(N, OW)
        o = (onz[:, 0:C] / onz[:, C:C + 1]).T         # [C, N]
        if g0 != 0.0:
            o = (1.0 - g0) * o + g0 * xf[b]
        out[b] = o.reshape(C, HH, WW)
    return out
